# revision 3
# baseline (speedup 1.0000x reference)
"""Trainium2 Bass kernel for nn_Model_11888469475981 (pooling) — v3.

Reference semantics per (n, c) sample-channel, padded row A[w'] (w'=0..258,
A = zeropad3d(x) rows along W):
  windows m=0..128 cover (A[2m], A[2m+1], A[2m+2]); M[m] = window max;
  position w' receives softsign(A[w']) iff it attains the max of a window
  covering it; fused = A + received*softsign(A); out = mean over padded D(17).

v3 restructure (all elementwise in fp16, dense deinterleaved blocks):
  OD[m] = A[2m+1], EV[i] = A[2i]  (ACT deinterleaves + converts f32->fp16)
  E[m] = max(EV[m], OD[m]); M[m] = max(E[m], EV[m+1])        (Pool engine)
  Mmin[i] = min(M[i-1], M[i])   (guards +BIG at edges)
  den = |M|+1 (ts2 @4x); ssM = M/den; ssmin[i] = min(ssM[i-1], ssM[i])
  mask = is_ge([OD|EV], [M|Mmin]); m2 = mask * [ssM|ssmin]
  psum[hg, :] = sum_d (VD + m2) * (1/17) via two fp16 matmuls per slot.
  (On ties both positions receive ss — values equal, error negligible.)

Output written deinterleaved [C, 64, 264] (od block cols 1..129, ev block
cols 132..261); the host reinterleaves into [C, 66, 259] and adds the
zero H-pad rows.

PSUM stacking: 8 channels share the psum banks at partition offset 8k
(k = c%8) via a shifted selector lhsT [128, 64]; groups of 8 alternate
between partition halves 0:64 / 64:128, one ACT evacuation per group.
"""

import numpy as np

import concourse.bass as bass
import concourse.mybir as mybir
from concourse import bacc
from concourse import library_config
from concourse.tile import TileContext
from concourse.bass_utils import run_bass_kernel_spmd

N_CORES = 8
C, D, H, W = 32, 16, 64, 256
NS = 8               # h-subslots per partition (h%8)
SLOT = 264           # slot width for both A (f32) and block tiles (fp16)
BLK = 132            # half-block width (od / ev)
BIG = 1000.0
F32 = mybir.dt.float32
F16 = mybir.dt.float16
Alu = mybir.AluOpType
Act = mybir.ActivationFunctionType


def build_nc():
    nc = bacc.Bacc()
    x_ext = nc.declare_dram_parameter("x", [C, D, H, W], F32, isOutput=False)
    ww_ext = nc.declare_dram_parameter("ww", [128, 128], F16, isOutput=False)
    out_ext = nc.declare_dram_parameter("out", [C, 64, SLOT], F32, isOutput=True)

    with TileContext(nc) as tc:
        with tc.tile_pool(name="main", bufs=1) as pool, \
             tc.tile_pool(name="psum", bufs=1, space="PSUM") as psum_pool:
            NB = 5
            NA = 6
            a_ts = [pool.tile([128, NS * SLOT], F32, tag=f"a{i}", name=f"a{i}")
                    for i in range(NA)]
            vd_ts = [pool.tile([128, NS * SLOT], F16, tag=f"vd{i}", name=f"vd{i}")
                     for i in range(NB)]
            th_ts = [pool.tile([128, NS * SLOT], F16, tag=f"th{i}", name=f"th{i}")
                     for i in range(NB)]
            ss_ts = [pool.tile([128, NS * SLOT], F16, tag=f"ss{i}", name=f"ss{i}")
                     for i in range(NB)]
            dd_ts = [pool.tile([128, NS * SLOT], F16, tag=f"dd{i}", name=f"dd{i}")
                     for i in range(NB)]
            m2_ts = [pool.tile([128, NS * SLOT], F16, tag=f"m2{i}", name=f"m2{i}")
                     for i in range(NB)]
            e_ts = [pool.tile([128, NS * BLK], F16, tag=f"e{i}", name=f"e{i}")
                    for i in range(NB)]
            den_ts = [pool.tile([128, NS * BLK], F16, tag=f"dn{i}", name=f"dn{i}")
                      for i in range(NB)]
            rr_ts = [pool.tile([128, NS * BLK], F16, tag=f"rr{i}", name=f"rr{i}")
                     for i in range(NB)]
            osb_ts = [pool.tile([64, NS * SLOT], F32, tag=f"ob{i}", name=f"ob{i}")
                      for i in range(2)]
            ww_t = pool.tile([128, 128], F16, tag="ww", name="ww")
            ml_t = pool.tile([128, 1], F32, tag="ml", name="ml")
            ps = psum_pool.tile([128, NS * 512], F32, tag="ps", name="ps")

            # one-time init: only the guard columns that are READ and carry
            # semantics.  A: cols 2 (w'=0), 259 (w'=257), 260 (w'=258) must
            # be 0 (deint sources).  TH: cols 0, 130, 131 must be +BIG
            # (Mmin edge guards).  All other guard cols feed only m2/psum
            # columns the host never harvests.
            for t in a_ts:
                av0 = t[:].rearrange("p (s w) -> p s w", s=NS)
                nc.vector.memset(av0[:, :, 2:3], 0.0)
                nc.vector.memset(av0[:, :, 259:261], 0.0)
            for t in th_ts:
                tv0 = t[:].rearrange("p (s w) -> p s w", s=NS)
                nc.vector.memset(tv0[:, :, 0:1], BIG)
                nc.vector.memset(tv0[:, :, 130:132], BIG)
                nc.vector.memset(tv0[:, :, 262:264], BIG)
            for t in vd_ts:
                vv0 = t[:].rearrange("p (s w) -> p s w", s=NS)
                nc.vector.memset(vv0[:, :, 0:1], 0.0)
                nc.vector.memset(vv0[:, :, 130:132], 0.0)
                nc.vector.memset(vv0[:, :, 262:264], 0.0)
            for t in ss_ts:
                sv0 = t[:].rearrange("p (s w) -> p s w", s=NS)
                nc.vector.memset(sv0[:, :, 262:264], 1.0)
            # select threshold: d = th - vd is +0 exactly on a win, else
            # >= the fp16 gap (>= 6e-8 even in subnormals)
            nc.vector.memset(ml_t[:], 3e-8)
            nc.sync.dma_start(out=ww_t[:], in_=ww_ext[:, :])
            # Pool-engine TensorTensor needs the standard GPSIMD library.
            nc.gpsimd.load_library(library_config.standard)

            def views(c):
                a_t = a_ts[c % NA]
                return dict(
                    k=c % 8, g=c // 8, pb=((c // 8) % 2) * 64,
                    av=a_t[:].rearrange("p (s w) -> p s w", s=NS),
                    a4=a_t[:].rearrange("p (s w2 two) -> p s w2 two",
                                        s=NS, two=2),
                    vd=vd_ts[c % NB], th=th_ts[c % NB], ss=ss_ts[c % NB],
                    dd=dd_ts[c % NB], m2=m2_ts[c % NB],
                    vdv=vd_ts[c % NB][:].rearrange("p (s w) -> p s w", s=NS),
                    thv=th_ts[c % NB][:].rearrange("p (s w) -> p s w", s=NS),
                    ssv=ss_ts[c % NB][:].rearrange("p (s w) -> p s w", s=NS),
                    m2v=m2_ts[c % NB][:].rearrange("p (s w) -> p s w", s=NS),
                    ev_=e_ts[c % NB][:].rearrange("p (s w) -> p s w", s=NS),
                    dnv=den_ts[c % NB][:].rearrange("p (s w) -> p s w", s=NS),
                    rrv=rr_ts[c % NB][:].rearrange("p (s w) -> p s w", s=NS),
                    wsl=ww_t[:, 64 - 8 * (c % 8):128 - 8 * (c % 8)],
                )

            def stage_a(c):
                """Load + deinterleave + VD-term matmuls for channel c."""
                v = views(c)
                nc.sync.dma_start(
                    out=v["av"][:, :, 3:259],
                    in_=bass.AP(x_ext, c * D * H * W,
                                [[2048, 128], [256, NS], [1, W]]),
                )
                # deinterleave + f32->fp16: OD on ACT, EV on Pool
                nc.scalar.activation(v["vdv"][:, :, 1:130],
                                     v["a4"][:, :, 1:130, 1], Act.Copy)
                nc.gpsimd.tensor_copy(v["vdv"][:, :, 132:262],
                                      v["a4"][:, :, 1:131, 0])
                # VD term accumulates as soon as the deints land
                for s in range(NS):
                    po = ps[v["pb"]:v["pb"] + 64, 512 * s:512 * s + SLOT]
                    nc.tensor.matmul(po, v["wsl"], v["vdv"][:, s, :],
                                     start=(v["k"] == 0), stop=False)

            def stage_b1(c):
                """Window max + Mmin; launch the ACT abs/recip chain."""
                v = views(c)
                vdv, thv = v["vdv"], v["thv"]
                # window max: E[m]=max(EV[m],OD[m]); M[m]=max(E,EV[m+1])
                nc.vector.tensor_tensor(v["ev_"][:, :, 1:130],
                                        vdv[:, :, 132:261],
                                        vdv[:, :, 1:130], Alu.max)
                nc.vector.tensor_tensor(thv[:, :, 1:130], v["ev_"][:, :, 1:130],
                                        vdv[:, :, 133:262], Alu.max)
                nc.vector.tensor_tensor(thv[:, :, 132:262], thv[:, :, 0:130],
                                        thv[:, :, 1:131], Alu.min)
                # 1/(|M|+1): Abs + Reciprocal(x+1) on ACT (one act table set
                # covers Copy/Abs/Reciprocal)
                nc.scalar.activation(v["dnv"][:, :, 0:BLK], thv[:, :, 0:BLK],
                                     Act.Abs)
                eng = nc.scalar
                eng.add_instruction(mybir.InstActivation(
                    name=nc.get_next_instruction_name(),
                    func=Act.Reciprocal,
                    ins=[eng.lower_ap(v["dnv"][:, :, 0:BLK]),
                         mybir.ImmediateValue(dtype=F32, value=1.0),
                         mybir.ImmediateValue(dtype=F32, value=1.0),
                         mybir.ImmediateValue(dtype=F32, value=0.0)],
                    outs=[eng.lower_ap(v["rrv"][:, :, 0:BLK])],
                ))

            def stage_b2(c):
                """ss values, masks, m2 + matmuls (+ evac at group end)."""
                v = views(c)
                thv, ssv = v["thv"], v["ssv"]
                nc.vector.tensor_tensor(ssv[:, :, 0:BLK], thv[:, :, 0:BLK],
                                        v["rrv"][:, :, 0:BLK], Alu.mult)
                nc.vector.tensor_tensor(ssv[:, :, 132:262], ssv[:, :, 0:130],
                                        ssv[:, :, 1:131], Alu.min)
                # masks = [vd >= th]; m2 = masks * ss (mult is Pool-legal —
                # park it there for ~70% of channels for load balance)
                nc.vector.tensor_tensor(v["dd"][:], v["vd"][:], v["th"][:],
                                        Alu.is_ge)
                # keep the drain short: the last group's m2 goes on the
                # (much faster) DVE so the final evac isn't gated on Pool
                mul_eng = nc.vector if (c % 16) < 3 or c >= 28 else nc.gpsimd
                mul_eng.tensor_tensor(v["m2"][:], v["dd"][:], v["ss"][:],
                                      Alu.mult)
                for s in range(NS):
                    po = ps[v["pb"]:v["pb"] + 64, 512 * s:512 * s + SLOT]
                    nc.tensor.matmul(po, v["wsl"], v["m2v"][:, s, :],
                                     start=False, stop=(v["k"] == 7))

            def stage_c(c):
                """Evacuate + store a finished psum group (k == 7 only).
                Emitted later than b2 so ACT's queue interleaves the next
                channels' abs/recip ahead of the (long-waiting) evac."""
                v = views(c)
                if v["k"] != 7:
                    return
                g = v["g"]
                osb = osb_ts[g % 2]
                ov = osb[:].rearrange("p (s w) -> p s w", s=NS)
                pv = ps[v["pb"]:v["pb"] + 64, :].rearrange(
                    "p (s w) -> p s w", s=NS)
                nc.scalar.activation(ov[:, :, :], pv[:, :, 0:SLOT], Act.Copy)
                # issue the output DMA from the ACT queue so it never
                # head-of-line blocks the input loads on the SP queue
                nc.scalar.dma_start(
                    out=bass.AP(out_ext, g * 8 * 64 * SLOT,
                                [[64 * SLOT, 8], [8 * SLOT, 8],
                                 [SLOT, NS], [1, SLOT]]),
                    in_=ov[:, :, :],
                )

            # software-pipelined emission: per-engine instruction streams
            # execute in emission order, so skew the stages so no engine's
            # stream head-of-line blocks on a cross-engine dependency:
            # a(c) || b1(c-LAG) || b2(c-LAG-1)
            LAG = 2
            for c in range(C + LAG + 1):
                if c < C:
                    stage_a(c)
                if 0 <= c - LAG < C:
                    stage_b1(c - LAG)
                if 0 <= c - LAG - 1 < C:
                    stage_b2(c - LAG - 1)
                    stage_c(c - LAG - 1)
    nc.finalize()
    return nc


_CACHE: dict = {}


def _get_nc():
    if "nc" not in _CACHE:
        _CACHE["nc"] = build_nc()
    return _CACHE["nc"]


def make_in_maps(x: np.ndarray):
    ww = np.zeros((128, 128), np.float16)
    ww[np.arange(128), 64 + np.arange(128) % 8] = np.float16(1.0 / 17.0)
    return [{"x": np.ascontiguousarray(x[i]), "ww": ww} for i in range(N_CORES)]


def kernel(**inputs) -> np.ndarray:
    x = np.ascontiguousarray(np.asarray(inputs["x"], dtype=np.float32))
    assert x.shape == (N_CORES, C, D, H, W), x.shape
    nc = _get_nc()
    res = run_bass_kernel_spmd(nc, make_in_maps(x), list(range(N_CORES)))
    outs = []
    for i in range(N_CORES):
        r = res.results[i]["out"]          # [C, 64, 264] f32, deinterleaved
        full = np.zeros((C, 66, 259), np.float32)
        full[:, 1:65, 1::2] = r[:, :, 1:130]      # odd w' = 2m+1
        full[:, 1:65, 0::2] = r[:, :, 132:262]    # even w' = 2i
        outs.append(full)
    return np.stack(outs, axis=0)


# revision 4
# speedup vs baseline: 1.0118x; 1.0118x over previous
"""Trainium2 Bass kernel for nn_Model_11888469475981 (pooling) — v3.

Reference semantics per (n, c) sample-channel, padded row A[w'] (w'=0..258,
A = zeropad3d(x) rows along W):
  windows m=0..128 cover (A[2m], A[2m+1], A[2m+2]); M[m] = window max;
  position w' receives softsign(A[w']) iff it attains the max of a window
  covering it; fused = A + received*softsign(A); out = mean over padded D(17).

v3 restructure (all elementwise in fp16, dense deinterleaved blocks):
  OD[m] = A[2m+1], EV[i] = A[2i]  (ACT deinterleaves + converts f32->fp16)
  E[m] = max(EV[m], OD[m]); M[m] = max(E[m], EV[m+1])        (Pool engine)
  Mmin[i] = min(M[i-1], M[i])   (guards +BIG at edges)
  den = |M|+1 (ts2 @4x); ssM = M/den; ssmin[i] = min(ssM[i-1], ssM[i])
  mask = is_ge([OD|EV], [M|Mmin]); m2 = mask * [ssM|ssmin]
  psum[hg, :] = sum_d (VD + m2) * (1/17) via two fp16 matmuls per slot.
  (On ties both positions receive ss — values equal, error negligible.)

Output written deinterleaved [C, 64, 264] (od block cols 1..129, ev block
cols 132..261); the host reinterleaves into [C, 66, 259] and adds the
zero H-pad rows.

PSUM stacking: 8 channels share the psum banks at partition offset 8k
(k = c%8) via a shifted selector lhsT [128, 64]; groups of 8 alternate
between partition halves 0:64 / 64:128, one ACT evacuation per group.
"""

import numpy as np

import concourse.bass as bass
import concourse.mybir as mybir
from concourse import bacc
from concourse import library_config
from concourse.tile import TileContext
from concourse.bass_utils import run_bass_kernel_spmd

N_CORES = 8
C, D, H, W = 32, 16, 64, 256
NS = 8               # h-subslots per partition (h%8)
SLOT = 264           # slot width for both A (f32) and block tiles (fp16)
BLK = 132            # half-block width (od / ev)
BIG = 1000.0
F32 = mybir.dt.float32
F16 = mybir.dt.float16
Alu = mybir.AluOpType
Act = mybir.ActivationFunctionType


def build_nc():
    nc = bacc.Bacc()
    x_ext = nc.declare_dram_parameter("x", [C, D, H, W], F32, isOutput=False)
    ww_ext = nc.declare_dram_parameter("ww", [128, 128], F16, isOutput=False)
    out_ext = nc.declare_dram_parameter("out", [C, 64, SLOT], F32, isOutput=True)

    with TileContext(nc) as tc:
        with tc.tile_pool(name="main", bufs=1) as pool, \
             tc.tile_pool(name="psum", bufs=1, space="PSUM") as psum_pool:
            NB = 5
            NA = 6
            a_ts = [pool.tile([128, NS * SLOT], F32, tag=f"a{i}", name=f"a{i}")
                    for i in range(NA)]
            vd_ts = [pool.tile([128, NS * SLOT], F16, tag=f"vd{i}", name=f"vd{i}")
                     for i in range(NB)]
            th_ts = [pool.tile([128, NS * SLOT], F16, tag=f"th{i}", name=f"th{i}")
                     for i in range(NB)]
            ss_ts = [pool.tile([128, NS * SLOT], F16, tag=f"ss{i}", name=f"ss{i}")
                     for i in range(NB)]
            dd_ts = [pool.tile([128, NS * SLOT], F16, tag=f"dd{i}", name=f"dd{i}")
                     for i in range(NB)]
            m2_ts = [pool.tile([128, NS * SLOT], F16, tag=f"m2{i}", name=f"m2{i}")
                     for i in range(NB)]
            e_ts = [pool.tile([128, NS * BLK], F16, tag=f"e{i}", name=f"e{i}")
                    for i in range(NB)]
            den_ts = [pool.tile([128, NS * BLK], F16, tag=f"dn{i}", name=f"dn{i}")
                      for i in range(NB)]
            rr_ts = [pool.tile([128, NS * BLK], F16, tag=f"rr{i}", name=f"rr{i}")
                     for i in range(NB)]
            osb_ts = [pool.tile([64, NS * SLOT], F32, tag=f"ob{i}", name=f"ob{i}")
                      for i in range(2)]
            ww_t = pool.tile([128, 128], F16, tag="ww", name="ww")
            ml_t = pool.tile([128, 1], F32, tag="ml", name="ml")
            ps = psum_pool.tile([128, NS * 512], F32, tag="ps", name="ps")

            # one-time init: only the guard columns that are READ and carry
            # semantics.  A: cols 2 (w'=0), 259 (w'=257), 260 (w'=258) must
            # be 0 (deint sources).  TH: cols 0, 130, 131 must be +BIG
            # (Mmin edge guards).  All other guard cols feed only m2/psum
            # columns the host never harvests.
            for t in a_ts:
                av0 = t[:].rearrange("p (s w) -> p s w", s=NS)
                nc.vector.memset(av0[:, :, 2:3], 0.0)
                nc.vector.memset(av0[:, :, 259:261], 0.0)
            for t in th_ts:
                tv0 = t[:].rearrange("p (s w) -> p s w", s=NS)
                nc.vector.memset(tv0[:, :, 0:1], BIG)
                nc.vector.memset(tv0[:, :, 130:132], BIG)
                nc.vector.memset(tv0[:, :, 262:264], BIG)
            for t in vd_ts:
                vv0 = t[:].rearrange("p (s w) -> p s w", s=NS)
                nc.vector.memset(vv0[:, :, 0:1], 0.0)
                nc.vector.memset(vv0[:, :, 130:132], 0.0)
                nc.vector.memset(vv0[:, :, 262:264], 0.0)
            for t in ss_ts:
                sv0 = t[:].rearrange("p (s w) -> p s w", s=NS)
                nc.vector.memset(sv0[:, :, 262:264], 1.0)
            # select threshold: d = th - vd is +0 exactly on a win, else
            # >= the fp16 gap (>= 6e-8 even in subnormals)
            nc.vector.memset(ml_t[:], 3e-8)
            nc.sync.dma_start(out=ww_t[:], in_=ww_ext[:, :])
            # Pool-engine TensorTensor needs the standard GPSIMD library.
            nc.gpsimd.load_library(library_config.standard)

            def views(c):
                a_t = a_ts[c % NA]
                return dict(
                    k=c % 8, g=c // 8, pb=((c // 8) % 2) * 64,
                    av=a_t[:].rearrange("p (s w) -> p s w", s=NS),
                    a4=a_t[:].rearrange("p (s w2 two) -> p s w2 two",
                                        s=NS, two=2),
                    vd=vd_ts[c % NB], th=th_ts[c % NB], ss=ss_ts[c % NB],
                    dd=dd_ts[c % NB], m2=m2_ts[c % NB],
                    vdv=vd_ts[c % NB][:].rearrange("p (s w) -> p s w", s=NS),
                    thv=th_ts[c % NB][:].rearrange("p (s w) -> p s w", s=NS),
                    ssv=ss_ts[c % NB][:].rearrange("p (s w) -> p s w", s=NS),
                    m2v=m2_ts[c % NB][:].rearrange("p (s w) -> p s w", s=NS),
                    ev_=e_ts[c % NB][:].rearrange("p (s w) -> p s w", s=NS),
                    dnv=den_ts[c % NB][:].rearrange("p (s w) -> p s w", s=NS),
                    rrv=rr_ts[c % NB][:].rearrange("p (s w) -> p s w", s=NS),
                    wsl=ww_t[:, 64 - 8 * (c % 8):128 - 8 * (c % 8)],
                )

            def stage_a(c):
                """Load + deinterleave + VD-term matmuls for channel c."""
                v = views(c)
                nc.sync.dma_start(
                    out=v["av"][:, :, 3:259],
                    in_=bass.AP(x_ext, c * D * H * W,
                                [[2048, 128], [256, NS], [1, W]]),
                )
                # deinterleave + f32->fp16: OD on ACT, EV on Pool
                nc.scalar.activation(v["vdv"][:, :, 1:130],
                                     v["a4"][:, :, 1:130, 1], Act.Copy)
                nc.gpsimd.tensor_copy(v["vdv"][:, :, 132:262],
                                      v["a4"][:, :, 1:131, 0])
                # VD term accumulates as soon as the deints land
                for s in range(NS):
                    po = ps[v["pb"]:v["pb"] + 64, 512 * s:512 * s + SLOT]
                    nc.tensor.matmul(po, v["wsl"], v["vdv"][:, s, :],
                                     start=(v["k"] == 0), stop=False)

            def stage_b1(c):
                """Window max + Mmin; launch the ACT abs/recip chain."""
                v = views(c)
                vdv, thv = v["vdv"], v["thv"]
                # window max: E[m]=max(EV[m],OD[m]); M[m]=max(E,EV[m+1])
                nc.vector.tensor_tensor(v["ev_"][:, :, 1:130],
                                        vdv[:, :, 132:261],
                                        vdv[:, :, 1:130], Alu.max)
                nc.vector.tensor_tensor(thv[:, :, 1:130], v["ev_"][:, :, 1:130],
                                        vdv[:, :, 133:262], Alu.max)
                nc.vector.tensor_tensor(thv[:, :, 132:262], thv[:, :, 0:130],
                                        thv[:, :, 1:131], Alu.min)
                # 1/(|M|+1): Abs + Reciprocal(x+1) on ACT (one act table set
                # covers Copy/Abs/Reciprocal)
                nc.vector.tensor_tensor(v["dd"][:], v["vd"][:], v["th"][:],
                                        Alu.is_ge)
                nc.scalar.activation(v["dnv"][:, :, 0:BLK], thv[:, :, 0:BLK],
                                     Act.Abs)
                eng = nc.scalar
                eng.add_instruction(mybir.InstActivation(
                    name=nc.get_next_instruction_name(),
                    func=Act.Reciprocal,
                    ins=[eng.lower_ap(v["dnv"][:, :, 0:BLK]),
                         mybir.ImmediateValue(dtype=F32, value=1.0),
                         mybir.ImmediateValue(dtype=F32, value=1.0),
                         mybir.ImmediateValue(dtype=F32, value=0.0)],
                    outs=[eng.lower_ap(v["rrv"][:, :, 0:BLK])],
                ))

            def stage_b2(c):
                """ss values, masks, m2 + matmuls (+ evac at group end)."""
                v = views(c)
                thv, ssv = v["thv"], v["ssv"]
                nc.vector.tensor_tensor(ssv[:, :, 0:BLK], thv[:, :, 0:BLK],
                                        v["rrv"][:, :, 0:BLK], Alu.mult)
                nc.vector.tensor_tensor(ssv[:, :, 132:262], ssv[:, :, 0:130],
                                        ssv[:, :, 1:131], Alu.min)
                # masks = [vd >= th]; m2 = masks * ss (mult is Pool-legal —
                # park it there for ~70% of channels for load balance)
                # keep the drain short: the last group's m2 goes on the
                # (much faster) DVE so the final evac isn't gated on Pool
                mul_eng = nc.vector if (c % 16) < 3 or c >= 28 else nc.gpsimd
                mul_eng.tensor_tensor(v["m2"][:], v["dd"][:], v["ss"][:],
                                      Alu.mult)
                for s in range(NS):
                    po = ps[v["pb"]:v["pb"] + 64, 512 * s:512 * s + SLOT]
                    nc.tensor.matmul(po, v["wsl"], v["m2v"][:, s, :],
                                     start=False, stop=(v["k"] == 7))

            def stage_c(c):
                """Evacuate + store a finished psum group (k == 7 only).
                Emitted later than b2 so ACT's queue interleaves the next
                channels' abs/recip ahead of the (long-waiting) evac."""
                v = views(c)
                if v["k"] != 7:
                    return
                g = v["g"]
                osb = osb_ts[g % 2]
                ov = osb[:].rearrange("p (s w) -> p s w", s=NS)
                pv = ps[v["pb"]:v["pb"] + 64, :].rearrange(
                    "p (s w) -> p s w", s=NS)
                nc.scalar.activation(ov[:, :, :], pv[:, :, 0:SLOT], Act.Copy)
                # issue the output DMA from the ACT queue so it never
                # head-of-line blocks the input loads on the SP queue
                nc.scalar.dma_start(
                    out=bass.AP(out_ext, g * 8 * 64 * SLOT,
                                [[64 * SLOT, 8], [8 * SLOT, 8],
                                 [SLOT, NS], [1, SLOT]]),
                    in_=ov[:, :, :],
                )

            # software-pipelined emission: per-engine instruction streams
            # execute in emission order, so skew the stages so no engine's
            # stream head-of-line blocks on a cross-engine dependency:
            # a(c) || b1(c-LAG) || b2(c-LAG-1)
            LAG = 2
            for c in range(C + LAG + 1):
                if c < C:
                    stage_a(c)
                if 0 <= c - LAG < C:
                    stage_b1(c - LAG)
                if 0 <= c - LAG - 1 < C:
                    stage_b2(c - LAG - 1)
                    stage_c(c - LAG - 1)
    nc.finalize()
    return nc


_CACHE: dict = {}


def _get_nc():
    if "nc" not in _CACHE:
        _CACHE["nc"] = build_nc()
    return _CACHE["nc"]


def make_in_maps(x: np.ndarray):
    ww = np.zeros((128, 128), np.float16)
    ww[np.arange(128), 64 + np.arange(128) % 8] = np.float16(1.0 / 17.0)
    return [{"x": np.ascontiguousarray(x[i]), "ww": ww} for i in range(N_CORES)]


def kernel(**inputs) -> np.ndarray:
    x = np.ascontiguousarray(np.asarray(inputs["x"], dtype=np.float32))
    assert x.shape == (N_CORES, C, D, H, W), x.shape
    nc = _get_nc()
    res = run_bass_kernel_spmd(nc, make_in_maps(x), list(range(N_CORES)))
    outs = []
    for i in range(N_CORES):
        r = res.results[i]["out"]          # [C, 64, 264] f32, deinterleaved
        full = np.zeros((C, 66, 259), np.float32)
        full[:, 1:65, 1::2] = r[:, :, 1:130]      # odd w' = 2m+1
        full[:, 1:65, 0::2] = r[:, :, 132:262]    # even w' = 2i
        outs.append(full)
    return np.stack(outs, axis=0)
